# revision 1
# baseline (speedup 1.0000x reference)
import sys
import time
import numpy as np

sys.path.insert(0, '/opt/trn_rl_repo')

from concourse import bass, bacc, mybir
from concourse.bass_utils import run_bass_kernel_spmd
from concourse.masks import make_identity
import concourse.tile as tile

# Problem constants (hardcoded per contract)
N = 260000
E = 8320000
GRAPH_NODES = 26
IN_DIM, H1, H2 = 4, 26, 11
POOL_OUT = 4
CORES = 8
NPC = N // CORES            # 32500 nodes per core
GPC = NPC // GRAPH_NODES    # 1250 graphs per core
F32 = mybir.dt.float32

_cache = {}
perf = {}


def _build_kernel_a(D1):
    """Per core: msg1 [NPC, 5*D1] -> m [NPC, 11].
    agg5 = reduce(msg1 view [*,5,D1], axis=-1); gcn1 = agg5 @ W1aug.T;
    h1 = tanh(gcn1); m = h1 @ W2.T
    """
    nc = bacc.Bacc("TRN2", target_bir_lowering=False, debug=False,
                   num_devices=CORES)
    msg = nc.dram_tensor("msg", [NPC, 5 * D1], F32, kind="ExternalInput")
    w1t = nc.dram_tensor("w1t", [5, H1], F32, kind="ExternalInput")
    w2t = nc.dram_tensor("w2t", [H1, H2], F32, kind="ExternalInput")
    m_out = nc.dram_tensor("m", [NPC, H2], F32, kind="ExternalOutput")

    P = 128
    n_tiles = (NPC + P - 1) // P
    with tile.TileContext(nc) as tc:
        with tc.tile_pool(name="const", bufs=1) as constp, \
             tc.tile_pool(name="msgp", bufs=4) as msgp, \
             tc.tile_pool(name="work", bufs=3) as work, \
             tc.tile_pool(name="psum", bufs=2, space="PSUM") as psum:
            ident = constp.tile([P, P], F32)
            make_identity(nc, ident[:])
            w1_t = constp.tile([5, H1], F32)
            nc.sync.dma_start(out=w1_t[:], in_=w1t[:, :])
            w2_t = constp.tile([H1, H2], F32)
            nc.sync.dma_start(out=w2_t[:], in_=w2t[:, :])

            for t in range(n_tiles):
                a = t * P
                b = min(a + P, NPC)
                p = b - a
                mt = msgp.tile([P, 5 * D1], F32, tag="mt")
                nc.sync.dma_start(out=mt[:p], in_=msg[a:b])
                agg5 = work.tile([P, 5], F32, tag="agg5")
                nc.vector.tensor_reduce(
                    out=agg5[:p],
                    in_=mt[:p].rearrange("p (c d) -> p c d", d=D1),
                    axis=mybir.AxisListType.X, op=mybir.AluOpType.add)
                agg5t_p = psum.tile([5, P], F32, tag="agg5t_p")
                nc.tensor.transpose(out=agg5t_p[:, :p], in_=agg5[:p],
                                    identity=ident[:p, :p])
                agg5t = work.tile([5, P], F32, tag="agg5t")
                nc.vector.tensor_copy(out=agg5t[:, :p], in_=agg5t_p[:, :p])
                gcn1_p = psum.tile([P, H1], F32, tag="gcn1_p")
                nc.tensor.matmul(out=gcn1_p[:p], lhsT=agg5t[:, :p],
                                 rhs=w1_t[:], start=True, stop=True)
                h1 = work.tile([P, H1], F32, tag="h1")
                nc.scalar.activation(out=h1[:p], in_=gcn1_p[:p],
                                     func=mybir.ActivationFunctionType.Tanh)
                h1t_p = psum.tile([H1, P], F32, tag="h1t_p")
                nc.tensor.transpose(out=h1t_p[:, :p], in_=h1[:p],
                                    identity=ident[:p, :p])
                h1t = work.tile([H1, P], F32, tag="h1t")
                nc.vector.tensor_copy(out=h1t[:, :p], in_=h1t_p[:, :p])
                m_p = psum.tile([P, H2], F32, tag="m_p")
                nc.tensor.matmul(out=m_p[:p], lhsT=h1t[:, :p], rhs=w2_t[:],
                                 start=True, stop=True)
                m_s = work.tile([P, H2], F32, tag="m_s")
                nc.vector.tensor_copy(out=m_s[:p], in_=m_p[:p])
                nc.sync.dma_start(out=m_out[a:b], in_=m_s[:p])
    nc.compile()
    return nc


def _build_kernel_b(D2):
    """Per core: msg2 [NPC, 11*D2] -> out [GPC + 2, 2] (last 2 rows junk).
    gcn2 = reduce; h2 = tanh; maxpool -> [*,4]; graph-sum over 26 nodes;
    z = g @ Wl.T + bl; softmax (2-class -> sigmoid of logit diff).
    """
    nc = bacc.Bacc("TRN2", target_bir_lowering=False, debug=False,
                   num_devices=CORES)
    msg = nc.dram_tensor("msg", [NPC, H2 * D2], F32, kind="ExternalInput")
    omat_d = nc.dram_tensor("omat", [104, 4], F32, kind="ExternalInput")
    dwb_d = nc.dram_tensor("dwb", [4, POOL_OUT + 1], F32, kind="ExternalInput")
    out_d = nc.dram_tensor("out", [GPC + 2, 2], F32, kind="ExternalOutput")

    P = 104  # 4 graphs of 26 nodes per tile
    n_tiles = (NPC + P - 1) // P  # 313; last tile 52 nodes (2 graphs)
    n_gt = 32
    with tile.TileContext(nc) as tc:
        with tc.tile_pool(name="const", bufs=1) as constp, \
             tc.tile_pool(name="msgp", bufs=4) as msgp, \
             tc.tile_pool(name="work", bufs=3) as work, \
             tc.tile_pool(name="gall", bufs=1) as gallp, \
             tc.tile_pool(name="gpsum", bufs=2, space="PSUM") as gpsum:
            omat = constp.tile([104, 4], F32)
            nc.sync.dma_start(out=omat[:], in_=omat_d[:, :])
            dwb = constp.tile([4, POOL_OUT + 1], F32)
            nc.sync.dma_start(out=dwb[:], in_=dwb_d[:, :])
            g_all = gallp.tile([4, n_tiles * 4], F32)

            gt = None
            for t in range(n_tiles):
                a = t * P
                b = min(a + P, NPC)
                p = b - a
                mt = msgp.tile([P, H2 * D2], F32, tag="mt")
                nc.sync.dma_start(out=mt[:p], in_=msg[a:b])
                gcn2 = work.tile([P, H2], F32, tag="gcn2")
                nc.vector.tensor_reduce(
                    out=gcn2[:p],
                    in_=mt[:p].rearrange("p (c d) -> p c d", d=D2),
                    axis=mybir.AxisListType.X, op=mybir.AluOpType.add)
                h2 = work.tile([P, H2], F32, tag="h2")
                nc.scalar.activation(out=h2[:p], in_=gcn2[:p],
                                     func=mybir.ActivationFunctionType.Tanh)
                pooled = work.tile([P, POOL_OUT], F32, tag="pooled")
                for j, (c0, c1) in enumerate([(0, 2), (2, 5), (5, 8), (8, 11)]):
                    nc.vector.tensor_reduce(out=pooled[:p, j:j + 1],
                                            in_=h2[:p, c0:c1],
                                            axis=mybir.AxisListType.X,
                                            op=mybir.AluOpType.max)
                if t % n_gt == 0:
                    gt = gpsum.tile([4, 4 * n_gt], F32, tag="gt")
                j = t % n_gt
                nc.tensor.matmul(out=gt[:, j * 4:(j + 1) * 4],
                                 lhsT=omat[:p], rhs=pooled[:p],
                                 start=True, stop=True)
                if j == n_gt - 1 or t == n_tiles - 1:
                    base = (t // n_gt) * n_gt * 4
                    w = (j + 1) * 4
                    nc.vector.tensor_copy(out=g_all[:, base:base + w],
                                          in_=gt[:, :w])

            # diff[p, t] = sum_c g_all[p, t*4+c]*dW[c] + db, probs via sigmoid
            diff = work.tile([4, n_tiles], F32, tag="diff")
            tmp = work.tile([4, n_tiles], F32, tag="tmp")
            for c in range(POOL_OUT):
                src = g_all[:, c::4]
                if c == 0:
                    nc.vector.tensor_scalar(out=diff[:], in0=src,
                                            scalar1=dwb[:, 0:1], scalar2=None,
                                            op0=mybir.AluOpType.mult)
                else:
                    nc.vector.tensor_scalar(out=tmp[:], in0=src,
                                            scalar1=dwb[:, c:c + 1], scalar2=None,
                                            op0=mybir.AluOpType.mult)
                    nc.vector.tensor_tensor(out=diff[:], in0=diff[:], in1=tmp[:],
                                            op=mybir.AluOpType.add)
            nc.vector.tensor_scalar(out=diff[:], in0=diff[:],
                                    scalar1=dwb[:, POOL_OUT:POOL_OUT + 1],
                                    scalar2=None, op0=mybir.AluOpType.add)
            s0 = work.tile([4, n_tiles], F32, tag="s0")
            s1 = work.tile([4, n_tiles], F32, tag="s1")
            nc.scalar.activation(out=s0[:], in_=diff[:],
                                 func=mybir.ActivationFunctionType.Sigmoid)
            nc.scalar.activation(out=s1[:], in_=diff[:],
                                 func=mybir.ActivationFunctionType.Sigmoid,
                                 scale=-1.0)
            ov = out_d[:, :].rearrange("(t p) o -> p t o", p=4)
            nc.sync.dma_start(out=ov[:, :, 0:1],
                              in_=s0[:].rearrange("p (t o) -> p t o", o=1))
            nc.sync.dma_start(out=ov[:, :, 1:2],
                              in_=s1[:].rearrange("p (t o) -> p t o", o=1))
    nc.compile()
    return nc


def _prep_structure(edge_index):
    row = np.asarray(edge_index[0], dtype=np.int64)
    col = np.asarray(edge_index[1], dtype=np.int64)
    cnt = np.bincount(col, minlength=N)
    D1 = int(cnt.max()) + 1          # +1 for self loop
    SRC = np.full((N, D1), N, dtype=np.int32)   # sentinel N -> zero row
    SRC[:, 0] = np.arange(N, dtype=np.int32)
    order = np.argsort(col, kind='stable')
    cs = col[order]
    rs = row[order].astype(np.int32)
    starts = np.concatenate([[0], np.cumsum(cnt)[:-1]])
    pos = np.arange(E, dtype=np.int64) - starts[cs]
    SRC[cs, pos + 1] = rs
    deg = (cnt + 1).astype(np.float32)
    return SRC, deg, D1


def kernel(x, edge_index, W1, b1, W2, b2, Wl, bl):
    x = np.asarray(x, dtype=np.float32)
    W1 = np.asarray(W1, np.float32); b1 = np.asarray(b1, np.float32)
    W2 = np.asarray(W2, np.float32); b2 = np.asarray(b2, np.float32)
    Wl = np.asarray(Wl, np.float32); bl = np.asarray(bl, np.float32)

    SRC, deg, D1 = _prep_structure(edge_index)
    D2 = D1 + 1

    if ('a', D1) not in _cache:
        _cache[('a', D1)] = _build_kernel_a(D1)
    if ('b', D2) not in _cache:
        _cache[('b', D2)] = _build_kernel_b(D2)
    nca = _cache[('a', D1)]
    ncb = _cache[('b', D2)]

    # ---- layer 1 on device ----
    x5 = np.concatenate([x, np.ones((N, 1), np.float32)], axis=1)
    x5s = np.vstack([x5, np.zeros((1, 5), np.float32)])
    w1aug = np.concatenate([W1, b1[:, None]], axis=1)    # [26, 5]
    w1t = np.ascontiguousarray(w1aug.T)                  # [5, 26]
    w2t = np.ascontiguousarray(W2.T)                     # [26, 11]

    in_maps_a = []
    for k in range(CORES):
        sl = SRC[k * NPC:(k + 1) * NPC]
        msg1 = np.ascontiguousarray(
            x5s[sl].transpose(0, 2, 1)).reshape(NPC, 5 * D1)
        in_maps_a.append({"msg": msg1, "w1t": w1t, "w2t": w2t})
    t0 = time.time()
    res_a = run_bass_kernel_spmd(nca, in_maps_a, list(range(CORES)))
    perf['a'] = time.time() - t0
    m_full = np.concatenate([res_a.results[k]["m"] for k in range(CORES)],
                            axis=0)                      # [N, 11]
    m_s = np.vstack([m_full, np.zeros((1, H2), np.float32)])

    # ---- layer 2 on device ----
    omat = np.zeros((104, 4), np.float32)
    omat[np.arange(104), np.arange(104) // GRAPH_NODES] = 1.0
    dW = Wl[0] - Wl[1]
    db = np.float32(bl[0] - bl[1])
    dwb = np.tile(np.concatenate([dW, [db]]).astype(np.float32), (4, 1))
    degb2 = deg[:, None] * b2[None, :]                   # [N, 11]
    in_maps_b = []
    for k in range(CORES):
        sl = SRC[k * NPC:(k + 1) * NPC]
        msg2 = np.empty((NPC, H2, D2), np.float32)
        msg2[:, :, :D1] = m_s[sl].transpose(0, 2, 1)
        msg2[:, :, D1] = degb2[k * NPC:(k + 1) * NPC]
        in_maps_b.append({"msg": msg2.reshape(NPC, H2 * D2), "omat": omat,
                          "dwb": dwb})
    t0 = time.time()
    res_b = run_bass_kernel_spmd(ncb, in_maps_b, list(range(CORES)))
    perf['b'] = time.time() - t0
    out = np.concatenate([res_b.results[k]["out"][:GPC]
                          for k in range(CORES)], axis=0)
    return out



# revision 10
# speedup vs baseline: 2.5113x; 2.5113x over previous
import sys
import time
import numpy as np

sys.path.insert(0, '/opt/trn_rl_repo')

from concourse import bass, bacc, mybir
from concourse import bass2jax
from concourse.bass_utils import run_bass_kernel_spmd
from concourse.masks import make_identity
import concourse.tile as tile

# ---- problem constants (hardcoded per contract) ----
N = 260000
E = 8320000
CORES = 8
NPC = N // CORES            # 32500 nodes (cols) per core / per row-bucket
TW = NPC + 1                # gather table width (sentinel zero col at NPC)
GRAPH_NODES = 26
IN_DIM, H1, H2 = 4, 26, 11
GPC = NPC // GRAPH_NODES    # 1250 graphs per core

CC = 416                    # cols per chunk (= 16 graphs)
NCH = 79                    # chunks per core (78 * 416 + 52)
LAST_CC = 52
BW = 432                    # boundary positions per chunk (417 padded to 16*27)
BWW = BW // 16
CE0 = 1872                  # default edge-slot capacity per (bucket, chunk)

F32 = mybir.dt.float32
I16 = mybir.dt.int16

_cache = {}
_static = {}
perf = {}


try:
    from numba import njit

    @njit("int32[::1](int32[::1], int64)", cache=False)
    def _occ(key, nk):
        cnt = np.zeros(nk, np.int32)
        out = np.empty(key.size, np.int32)
        for e in range(key.size):
            kk = key[e]
            out[e] = cnt[kk]
            cnt[kk] += 1
        return out

    @njit("void(int32[::1], int32[::1], int32[::1])", cache=False)
    def _count(row, col, counts):
        npc = NPC
        for e in range(row.size):
            counts[(row[e] // npc * 8 + col[e] // npc) * npc
                   + col[e] % npc] += 1

    @njit("void(int32[::1], int32[::1], int32[::1], int32[::1], int32[::1], "
          "int16[::1], int64)", cache=False)
    def _fill(row, col, basek, occ_cnt, _unused, gidx_flat, gw):
        npc = NPC
        nch = NCH
        ccw = CC
        for e in range(row.size):
            r = row[e]
            c = col[e]
            b = r // npc
            rl = r - b * npc
            k = c // npc
            lc = c - k * npc
            key = (b * 8 + k) * npc + lc
            ch = lc // ccw
            if ch > nch - 1:
                ch = nch - 1
            i = basek[key] + occ_cnt[key] + 1
            occ_cnt[key] += 1
            p = 16 * b + (i & 15)
            gidx_flat[(k * 128 + p) * (nch * gw) + ch * gw + (i >> 4)] = rl
except Exception:                                 # pragma: no cover
    _occ = None
    _count = None
    _fill = None


def _get_static():
    if _static:
        return _static
    lcol = np.arange(NPC)
    chunk_of_lcol = np.minimum(lcol // CC, NCH - 1).astype(np.int32)
    # flat (b, col)-space start index of each cell, ordered (b, k, c)
    base_c = np.minimum(np.arange(NCH) * CC, NPC - LAST_CC)
    width_c = np.full(NCH, CC); width_c[NCH - 1] = LAST_CC
    starts = (np.arange(8)[:, None, None] * N
              + np.arange(8)[None, :, None] * NPC
              + base_c[None, None, :])           # [8b, 8k, 79]
    cell_col_starts = starts.reshape(-1).astype(np.int64)
    # boundary gather grid [79, BW] into per-(b,k) exclusive-cumsum (len NPC+1)
    j = np.arange(BW)
    idxgrid = base_c[:, None] + np.minimum(j[None, :], width_c[:, None])
    # per-key chunk id (for the flat key space (b*8+k)*NPC + lcol)
    _static['chunk_of_lcol'] = chunk_of_lcol
    _static['cell_col_starts'] = cell_col_starts
    _static['widths'] = np.diff(np.append(cell_col_starts, 8 * N))
    _static['idxgrid'] = idxgrid.astype(np.int64)
    _static['base_c'] = base_c.astype(np.int64)
    return _static


def _prep(edge_index, CE):
    """Build per-core gather-index / boundary arrays.

    Returns GIDX [8, 128, NCH*GW] int16, BND [8, 128, NCH*BWW] int16,
    deg fp32 [8, NPC], maxcell (edges in fullest cell).
    """
    st = _get_static()
    GW = CE // 16
    row = np.ascontiguousarray(edge_index[0]).astype(np.int32, copy=False)
    col = np.ascontiguousarray(edge_index[1]).astype(np.int32, copy=False)
    if not row.flags.writeable:
        row = row.copy()
    if not col.flags.writeable:
        col = col.copy()
    if _count is not None:
        counts = np.zeros(8 * N, np.int32)
        _count(row, col, counts)
    else:
        b0 = row // NPC
        k0 = col // NPC
        key0 = (b0 * 8 + k0) * NPC + (col - k0 * NPC)
        counts = np.bincount(key0, minlength=8 * N).astype(np.int32)
    cellcnt = np.add.reduceat(counts, st['cell_col_starts'])
    maxcell = int(cellcnt.max())
    if maxcell + 1 > CE:
        return None, None, None, maxcell

    # exclusive cumsum over lcol per (b, k); same flat indexing as key
    cnt3 = counts.reshape(8, 8, NPC)
    Bex = np.zeros((8, 8, NPC + 1), np.int32)
    np.cumsum(cnt3, axis=2, out=Bex[:, :, 1:], dtype=np.int32)
    BexK = np.ascontiguousarray(Bex[:, :, :NPC]).reshape(-1)   # value at key
    # in-cell col base offset per key
    cellbase = BexK[st['cell_col_starts']]
    basek = BexK - np.repeat(cellbase, st['widths'])

    GIDX = np.full(8 * 128 * NCH * GW, NPC, np.int16)
    if _fill is not None:
        occ_cnt = np.zeros(8 * N, np.int32)
        _fill(row, col, basek, occ_cnt, basek, GIDX, GW)
    else:
        b = row // NPC
        k = col // NPC
        lcol = col - k * NPC
        key = (b * 8 + k) * NPC + lcol
        c_e = st['chunk_of_lcol'][lcol]
        order = np.argsort(key, kind='stable')
        rank = np.empty(E, np.int32)
        ks = key[order]
        newrun = np.empty(E, bool)
        newrun[0] = True
        np.not_equal(ks[1:], ks[:-1], out=newrun[1:])
        idxs = np.arange(E, dtype=np.int64)
        runstart = np.maximum.accumulate(np.where(newrun, idxs, 0))
        rank[order] = (idxs - runstart).astype(np.int32)
        i = (basek[key] + rank + 1).astype(np.int64)
        p = 16 * b + (i & 15)
        flat = ((k * 128 + p) * (NCH * GW) + c_e * GW + (i >> 4)).astype(np.int64)
        GIDX[flat] = (row - b * NPC).astype(np.int16)
    GIDX = GIDX.reshape(8, 128, NCH * GW)

    Bc = Bex[:, :, st['idxgrid']] - Bex[:, :, st['base_c']][:, :, :, None]
    BND = (Bc.reshape(8, 8, NCH, BWW, 16)
             .transpose(1, 0, 4, 2, 3)
             .reshape(8, 128, NCH * BWW).astype(np.int16))

    deg = (cnt3.sum(axis=0) + 1).astype(np.float32)            # [8, NPC]
    return GIDX, BND, deg, maxcell


def _make_consts(W1, b1, W2, b2, Wl, bl):
    cst = np.zeros((128, 96), np.float32)
    W1aug = np.concatenate([W1, b1[:, None]], axis=1)          # [26, 5]
    cst[0:5, 0:26] = W1aug.T
    cst[0:26, 26:37] = W2.T
    for g in range(8):
        for f in range(4):
            cst[16 * g + f, 37 + f] = 1.0                      # mask1
        for f in range(11):
            cst[16 * g + f, 42 + f] = 1.0                      # mask2
    cst[0:5, 53:58] = np.eye(5)                                # I5
    r = np.arange(104)
    cst[r, 58 + r // 26] = 1.0                                 # omat104
    r = np.arange(52)
    cst[r, 62 + r // 26] = 1.0                                 # omat52
    dW = (Wl[0] - Wl[1]).astype(np.float32)
    db = np.float32(bl[0] - bl[1])
    dwb = np.concatenate([dW, [db]])
    cst[0:4, 64:69] = np.tile(dwb, (4, 1))                     # dwb4
    cst[0:2, 69:74] = np.tile(dwb, (2, 1))                     # dwb2
    cst[0:11, 74:85] = np.eye(11)                              # I11b2 rows 0-10
    cst[11, 74:85] = b2
    return cst


def _build_kernel(CE):
    GW = CE // 16
    nc = bacc.Bacc("TRN2", target_bir_lowering=False, debug=False,
                   num_devices=CORES)
    gidx_d = nc.dram_tensor("gidx", [128, NCH * GW], I16, kind="ExternalInput")
    bnd_d = nc.dram_tensor("bnd", [128, NCH * BWW], I16, kind="ExternalInput")
    xtd_d = nc.dram_tensor("xtd", [5, NPC], F32, kind="ExternalInput")
    cst_d = nc.dram_tensor("cst", [128, 96], F32, kind="ExternalInput")
    out_d = nc.dram_tensor("out", [GPC, 2], F32, kind="ExternalOutput")

    AG = "AllGather"
    BYP = mybir.AluOpType.bypass
    ADD = mybir.AluOpType.add
    SUB = mybir.AluOpType.subtract
    MULT = mybir.AluOpType.mult
    MAX = mybir.AluOpType.max
    TANH = mybir.ActivationFunctionType.Tanh
    COPY = mybir.ActivationFunctionType.Copy
    SIGM = mybir.ActivationFunctionType.Sigmoid
    XAX = mybir.AxisListType.X

    with tile.TileContext(nc) as tc:
        with tc.tile_pool(name="const", bufs=1) as cp, \
             tc.tile_pool(name="one", bufs=1) as onep, \
             tc.tile_pool(name="stream", bufs=2) as sp, \
             tc.tile_pool(name="dram", bufs=1, space="DRAM") as dp:
            cst = cp.tile([128, 96], F32)
            nc.sync.dma_start(out=cst[:], in_=cst_d[:, :])
            id11 = cp.tile([11, 11], F32)
            make_identity(nc, id11[:])
            # unpack small constants into dedicated tiles
            w1t = cp.tile([5, 26], F32)
            nc.vector.tensor_copy(out=w1t[:], in_=cst[0:5, 0:26])
            w2t = cp.tile([26, 11], F32)
            nc.vector.tensor_copy(out=w2t[:], in_=cst[0:26, 26:37])
            mask1 = cp.tile([128, 5], F32)
            nc.vector.tensor_copy(out=mask1[:], in_=cst[:, 37:42])
            mask2 = cp.tile([128, 11], F32)
            nc.vector.tensor_copy(out=mask2[:], in_=cst[:, 42:53])
            i5 = cp.tile([5, 5], F32)
            nc.vector.tensor_copy(out=i5[:], in_=cst[0:5, 53:58])
            om104 = cp.tile([104, 4], F32)
            nc.vector.tensor_copy(out=om104[:], in_=cst[0:104, 58:62])
            om52 = cp.tile([52, 2], F32)
            nc.vector.tensor_copy(out=om52[:], in_=cst[0:52, 62:64])
            dwb4 = cp.tile([4, 5], F32)
            nc.vector.tensor_copy(out=dwb4[:], in_=cst[0:4, 64:69])
            dwb2 = cp.tile([2, 5], F32)
            nc.vector.tensor_copy(out=dwb2[:], in_=cst[0:2, 69:74])
            i11b2 = cp.tile([12, 11], F32)
            nc.vector.tensor_copy(out=i11b2[:], in_=cst[0:12, 74:85])

            # DRAM internals
            xb = dp.tile([5, NPC], F32)
            xall = dp.tile([40, NPC], F32)
            mtd = dp.tile([11, NPC], F32)
            mall = dp.tile([88, NPC], F32)
            nc.sync.dma_start(out=xb[:], in_=xtd_d[:, :])
            nc.gpsimd.collective_compute(
                AG, BYP, replica_groups=[list(range(CORES))],
                ins=[xb[:].opt()], outs=[xall[:].opt()])

            gall = onep.tile([4, 1248], F32)
            gallb = onep.tile([2, 4], F32)

            def stream_chunk(c, tab):
                """gather -> scan -> boundary gather -> diff; returns acc."""
                cc = CC if c < NCH - 1 else LAST_CC
                gi = sp.tile([128, GW], I16, tag="gi")
                nc.sync.dma_start(out=gi[:], in_=gidx_d[:, c * GW:(c + 1) * GW])
                bn = sp.tile([128, BWW], I16, tag="bn")
                nc.sync.dma_start(out=bn[:], in_=bnd_d[:, c * BWW:(c + 1) * BWW])
                msg = sp.tile([128, CE], F32, tag="msg")
                nc.gpsimd.ap_gather(
                    out_ap=msg[:], in_ap=tab[:], idxs_ap=gi[:],
                    channels=128, num_elems=TW, d=1, num_idxs=CE)
                pref = onep.tile([128, CE], F32, tag="pref")
                nc.vector.tensor_tensor_scan(
                    out=pref[:], data0=msg[:], data1=msg[:], initial=0.0,
                    op0=ADD, op1=BYP)
                G = sp.tile([128, BW], F32, tag="G")
                nc.gpsimd.ap_gather(
                    out_ap=G[:], in_ap=pref[:], idxs_ap=bn[:],
                    channels=128, num_elems=CE, d=1, num_idxs=BW)
                acc = sp.tile([128, CC], F32, tag="acc")
                nc.vector.tensor_tensor(out=acc[:, :cc], in0=G[:, 1:cc + 1],
                                        in1=G[:, 0:cc], op=SUB)
                return acc, cc

            # ---------------- layer 1 ----------------
            with tc.tile_pool(name="tab1", bufs=1) as tp1, \
                 tc.tile_pool(name="ps1", bufs=2, space="PSUM") as ps:
                tab = tp1.tile([128, TW], F32)
                nc.vector.memset(tab[:], 0.0)
                for g in range(8):
                    nc.sync.dma_start(out=tab[16 * g:16 * g + 4, 0:NPC],
                                      in_=xall[5 * g:5 * g + 4, :])
                for c in range(NCH):
                    acc, cc = stream_chunk(c, tab)
                    xd = sp.tile([5, CC], F32, tag="xd")
                    nc.sync.dma_start(out=xd[:, :cc],
                                      in_=xtd_d[:, c * CC:c * CC + cc])
                    ag5 = ps.tile([5, CC], F32, tag="ag5")
                    nc.tensor.matmul(out=ag5[:, :cc], lhsT=mask1[:],
                                     rhs=acc[:, :cc], start=True, stop=False)
                    nc.tensor.matmul(out=ag5[:, :cc], lhsT=i5[:],
                                     rhs=xd[:, :cc], start=False, stop=True)
                    rhs5 = sp.tile([5, CC], F32, tag="rhs5")
                    nc.scalar.activation(out=rhs5[:, :cc], in_=ag5[:, :cc],
                                         func=COPY)
                    h1p = ps.tile([26, CC], F32, tag="h1p")
                    nc.tensor.matmul(out=h1p[:, :cc], lhsT=w1t[:],
                                     rhs=rhs5[:, :cc], start=True, stop=True)
                    h1s = sp.tile([26, CC], F32, tag="h1s")
                    nc.scalar.activation(out=h1s[:, :cc], in_=h1p[:, :cc],
                                         func=TANH)
                    mp = ps.tile([11, CC], F32, tag="mp")
                    nc.tensor.matmul(out=mp[:, :cc], lhsT=w2t[:],
                                     rhs=h1s[:, :cc], start=True, stop=True)
                    ms = sp.tile([11, CC], F32, tag="ms")
                    nc.scalar.activation(out=ms[:, :cc], in_=mp[:, :cc],
                                         func=COPY)
                    nc.sync.dma_start(out=mtd[:, c * CC:c * CC + cc],
                                      in_=ms[:, :cc])

            nc.gpsimd.collective_compute(
                AG, BYP, replica_groups=[list(range(CORES))],
                ins=[mtd[:].opt()], outs=[mall[:].opt()])

            # ---------------- layer 2 ----------------
            with tc.tile_pool(name="tab2", bufs=1) as tp2, \
                 tc.tile_pool(name="ps2", bufs=2, space="PSUM") as ps:
                tab2 = tp2.tile([128, TW], F32)
                nc.vector.memset(tab2[:], 0.0)
                for g in range(8):
                    nc.sync.dma_start(out=tab2[16 * g:16 * g + 11, 0:NPC],
                                      in_=mall[11 * g:11 * g + 11, :])
                for c in range(NCH):
                    acc, cc = stream_chunk(c, tab2)
                    md = sp.tile([12, CC], F32, tag="md")
                    nc.sync.dma_start(out=md[0:11, :cc],
                                      in_=mtd[:, c * CC:c * CC + cc])
                    nc.sync.dma_start(out=md[11:12, :cc],
                                      in_=xtd_d[4:5, c * CC:c * CC + cc])
                    ag11 = ps.tile([11, CC], F32, tag="ag11")
                    nc.tensor.matmul(out=ag11[:, :cc], lhsT=mask2[:],
                                     rhs=acc[:, :cc], start=True, stop=False)
                    nc.tensor.matmul(out=ag11[:, :cc], lhsT=i11b2[:],
                                     rhs=md[:, :cc], start=False, stop=True)
                    h2 = sp.tile([11, CC], F32, tag="h2")
                    nc.scalar.activation(out=h2[:, :cc], in_=ag11[:, :cc],
                                         func=TANH)
                    ntile = 4 if c < NCH - 1 else 1
                    tw_ = 104 if c < NCH - 1 else 52
                    for t in range(ntile):
                        trp = ps.tile([104, 11], F32, tag="trp")
                        nc.tensor.transpose(
                            out=trp[:tw_, :],
                            in_=h2[:, t * tw_:(t + 1) * tw_],
                            identity=id11[:])
                        ts = sp.tile([104, 12], F32, tag="ts")
                        nc.vector.memset(ts[:tw_, 0:1], -1e30)
                        nc.scalar.activation(out=ts[:tw_, 1:12],
                                             in_=trp[:tw_, :], func=COPY)
                        pool = sp.tile([104, 4], F32, tag="pool")
                        nc.vector.tensor_reduce(
                            out=pool[:tw_, :],
                            in_=ts[:tw_, :].rearrange("p (g w) -> p g w", w=3),
                            axis=XAX, op=MAX)
                        gt = ps.tile([4, 4], F32, tag="gt")
                        if c < NCH - 1:
                            nc.tensor.matmul(out=gt[0:4, :], lhsT=om104[:],
                                             rhs=pool[:tw_, :],
                                             start=True, stop=True)
                            T = 4 * c + t
                            nc.vector.tensor_copy(
                                out=gall[:, 4 * T:4 * T + 4], in_=gt[0:4, :])
                        else:
                            nc.tensor.matmul(out=gt[0:2, :], lhsT=om52[:],
                                             rhs=pool[:tw_, :],
                                             start=True, stop=True)
                            nc.vector.tensor_copy(out=gallb[:, :],
                                                  in_=gt[0:2, :])

                # ---- final linear + softmax (2-class sigmoid trick) ----
                diff = onep.tile([4, 312], F32, tag="diff")
                tmp = onep.tile([4, 312], F32, tag="tmp")
                for f in range(4):
                    src = gall[:, f::4]
                    if f == 0:
                        nc.vector.tensor_scalar(out=diff[:], in0=src,
                                                scalar1=dwb4[:, 0:1],
                                                scalar2=None, op0=MULT)
                    else:
                        nc.vector.tensor_scalar(out=tmp[:], in0=src,
                                                scalar1=dwb4[:, f:f + 1],
                                                scalar2=None, op0=MULT)
                        nc.vector.tensor_tensor(out=diff[:], in0=diff[:],
                                                in1=tmp[:], op=ADD)
                nc.vector.tensor_scalar(out=diff[:], in0=diff[:],
                                        scalar1=dwb4[:, 4:5], scalar2=None,
                                        op0=ADD)
                s0 = onep.tile([4, 312], F32, tag="s0")
                s1 = onep.tile([4, 312], F32, tag="s1")
                nc.scalar.activation(out=s0[:], in_=diff[:], func=SIGM)
                nc.scalar.activation(out=s1[:], in_=diff[:], func=SIGM,
                                     scale=-1.0)
                ov = out_d[0:1248, :].rearrange("(t p) o -> p t o", p=4)
                nc.sync.dma_start(out=ov[:, :, 0:1],
                                  in_=s0[:].rearrange("p (t o) -> p t o", o=1))
                nc.sync.dma_start(out=ov[:, :, 1:2],
                                  in_=s1[:].rearrange("p (t o) -> p t o", o=1))

                diffb = onep.tile([2, 1], F32, tag="diffb")
                tmpb = onep.tile([2, 1], F32, tag="tmpb")
                for f in range(4):
                    src = gallb[:, f:f + 1]
                    if f == 0:
                        nc.vector.tensor_scalar(out=diffb[:], in0=src,
                                                scalar1=dwb2[:, 0:1],
                                                scalar2=None, op0=MULT)
                    else:
                        nc.vector.tensor_scalar(out=tmpb[:], in0=src,
                                                scalar1=dwb2[:, f:f + 1],
                                                scalar2=None, op0=MULT)
                        nc.vector.tensor_tensor(out=diffb[:], in0=diffb[:],
                                                in1=tmpb[:], op=ADD)
                nc.vector.tensor_scalar(out=diffb[:], in0=diffb[:],
                                        scalar1=dwb2[:, 4:5], scalar2=None,
                                        op0=ADD)
                s0b = onep.tile([2, 1], F32, tag="s0b")
                s1b = onep.tile([2, 1], F32, tag="s1b")
                nc.scalar.activation(out=s0b[:], in_=diffb[:], func=SIGM)
                nc.scalar.activation(out=s1b[:], in_=diffb[:], func=SIGM,
                                     scale=-1.0)
                ovb = out_d[1248:1250, :].rearrange("(t p) o -> p t o", p=2)
                nc.sync.dma_start(out=ovb[:, :, 0:1],
                                  in_=s0b[:].rearrange("p (t o) -> p t o", o=1))
                nc.sync.dma_start(out=ovb[:, :, 1:2],
                                  in_=s1b[:].rearrange("p (t o) -> p t o", o=1))
    nc.compile()
    return nc


def _make_runner(nc):
    """Build the sharded jitted executor once (same path as
    bass2jax.run_bass_via_pjrt, but cached so repeat calls skip re-trace)."""
    import jax
    from jax.experimental.shard_map import shard_map
    from jax.sharding import Mesh, PartitionSpec

    bass2jax.install_neuronx_cc_hook()
    partition_name = (nc.partition_id_tensor.name
                      if nc.partition_id_tensor else None)
    in_names, out_names, out_avals, zero_outs = [], [], [], []
    for alloc in nc.m.functions[0].allocations:
        if not isinstance(alloc, mybir.MemoryLocationSet):
            continue
        name = alloc.memorylocations[0].name
        if alloc.kind == "ExternalInput":
            if name != partition_name:
                in_names.append(name)
        elif alloc.kind == "ExternalOutput":
            shape = tuple(alloc.tensor_shape)
            dtype = mybir.dt.np(alloc.dtype)
            out_names.append(name)
            out_avals.append(jax.core.ShapedArray(shape, dtype))
            zero_outs.append(np.zeros(shape, dtype))
    n_params = len(in_names)
    n_outs = len(out_avals)
    all_names = list(in_names) + list(out_names)
    if partition_name is not None:
        all_names.append(partition_name)
    donate = tuple(range(n_params, n_params + n_outs))

    def _body(*args):
        operands = list(args)
        if partition_name is not None:
            operands.append(bass2jax.partition_id_tensor())
        outs = bass2jax._bass_exec_p.bind(
            *operands,
            out_avals=tuple(out_avals),
            in_names=tuple(all_names),
            out_names=tuple(out_names),
            lowering_input_output_aliases=(),
            sim_require_finite=True,
            sim_require_nnan=True,
            nc=nc,
        )
        return tuple(outs)

    devices = jax.devices()[:CORES]
    mesh = Mesh(np.asarray(devices), ("core",))
    in_specs = (PartitionSpec("core"),) * (n_params + n_outs)
    out_specs = (PartitionSpec("core"),) * n_outs
    sharded = jax.jit(
        shard_map(_body, mesh=mesh, in_specs=in_specs, out_specs=out_specs,
                  check_rep=False),
        donate_argnums=donate, keep_unused=True)

    def run(in_maps):
        concat_in = [
            np.concatenate([np.asarray(m[name]) for m in in_maps], axis=0)
            for name in in_names]
        concat_zeros = [
            np.zeros((CORES * z.shape[0], *z.shape[1:]), z.dtype)
            for z in zero_outs]
        out_arrs = sharded(*concat_in, *concat_zeros)
        return [
            {name: np.asarray(out_arrs[i]).reshape(
                CORES, *out_avals[i].shape)[c]
             for i, name in enumerate(out_names)}
            for c in range(CORES)]
    return run


def kernel(x, edge_index, W1, b1, W2, b2, Wl, bl):
    x = np.asarray(x, np.float32)
    edge_index = np.asarray(edge_index)
    W1 = np.asarray(W1, np.float32); b1 = np.asarray(b1, np.float32)
    W2 = np.asarray(W2, np.float32); b2 = np.asarray(b2, np.float32)
    Wl = np.asarray(Wl, np.float32); bl = np.asarray(bl, np.float32)
    t0 = time.time()
    CE = CE0
    GIDX, BND, deg, maxcell = _prep(edge_index, CE)
    while GIDX is None:
        CE = ((maxcell + 1 + 15) // 16 + 3) * 16  # headroom, mult of 16
        GIDX, BND, deg, maxcell = _prep(edge_index, CE)
    perf['prep'] = time.time() - t0

    if CE not in _cache:
        nc = _build_kernel(CE)
        _cache[CE] = (nc, _make_runner(nc))
    nc, runner = _cache[CE]

    cst = _make_consts(W1, b1, W2, b2, Wl, bl)
    xT = np.ascontiguousarray(x.T)
    in_maps = []
    for k in range(CORES):
        xtd = np.concatenate([xT[:, k * NPC:(k + 1) * NPC], deg[k][None]],
                             axis=0).astype(np.float32)
        in_maps.append({"gidx": np.ascontiguousarray(GIDX[k]),
                        "bnd": np.ascontiguousarray(BND[k]),
                        "xtd": np.ascontiguousarray(xtd),
                        "cst": cst})
    t0 = time.time()
    results = runner(in_maps)
    perf['run'] = time.time() - t0
    out = np.concatenate([results[k]["out"] for k in range(CORES)], axis=0)
    return out
